# revision 17
# baseline (speedup 1.0000x reference)
"""Trainium2 Bass kernel for nn_CrossModalMoELayer.

Sharding: data-parallel over batch B=8 -> one batch element per NeuronCore.
Each core runs the full layer (self-attn, cross-attn, gating, dense MoE)
for its batch element; weights replicated, pre-transposed on host so the
PE gets stationary/moving operands in natural layout.

Layouts per core:
  tm = token-major  [tokens on partitions, features on free]
  fm = feature-major[features on partitions, tokens on free]
Matmuls contract over the partition dim -> feature contractions use fm
activations; per-token scalars (LN stats, softmax denoms, expert probs)
apply in tm as per-partition scalars. Attention softmax runs max-free
(logit range is ~[-2,2]); the denominator falls out of the attn@V matmul
via a ones-column appended to V. MoE expert mixture accumulates with one
fused scalar_tensor_tensor per (expert, token-chunk); the residual is
folded in at expert 0.
"""

import numpy as np
from contextlib import ExitStack

import concourse.bass as bass
import concourse.bacc as bacc
import concourse.tile as tile
from concourse import mybir
from concourse.masks import make_identity
from concourse.bass_utils import run_bass_kernel_spmd

F32 = mybir.dt.float32
F32R = mybir.dt.float32r
AF = mybir.ActivationFunctionType
ALU = mybir.AluOpType
AX = mybir.AxisListType

B, SQ, SI, ST = 8, 512, 1024, 256
H, NH, E, I = 512, 8, 8, 1024
DH = H // NH          # 64
EPS = 1e-5
P = 128
HC = H // P           # 4
SQC = SQ // P         # 4
SIC = SI // P         # 8
STC = ST // P         # 2
IC = I // P           # 8
SAC = (SQ + SI) // P  # 12


def r(ap):
    return ap.bitcast(F32R)


def build_nc():
    nc = bacc.Bacc("TRN2", target_bir_lowering=False, debug=False,
                   enable_asserts=False)

    xq_d = nc.dram_tensor("xq", [SQ, H], F32, kind="ExternalInput")
    xi_d = nc.dram_tensor("xi", [SI, H], F32, kind="ExternalInput")
    xt_d = nc.dram_tensor("xt", [ST, H], F32, kind="ExternalInput")
    ln_d = {}
    for nm in ["lnq_g", "lnq_b", "lnc_g", "lnc_b", "lnf_g", "lnf_b"]:
        ln_d[nm] = nc.dram_tensor(nm, [H], F32, kind="ExternalInput")
    wsa_qkvT_d = nc.dram_tensor("wsa_qkvT", [H, 3 * H], F32, kind="ExternalInput")
    bsa_qkv_d = nc.dram_tensor("bsa_qkv", [3 * H], F32, kind="ExternalInput")
    wsa_outT_d = nc.dram_tensor("wsa_outT", [H, H], F32, kind="ExternalInput")
    bsa_out_d = nc.dram_tensor("bsa_out", [H], F32, kind="ExternalInput")
    wca_qkvT_d = nc.dram_tensor("wca_qkvT", [H, 3 * H], F32, kind="ExternalInput")
    bca_qkv_d = nc.dram_tensor("bca_qkv", [3 * H], F32, kind="ExternalInput")
    wca_outT_d = nc.dram_tensor("wca_outT", [H, H], F32, kind="ExternalInput")
    bca_out_d = nc.dram_tensor("bca_out", [H], F32, kind="ExternalInput")
    ig_wT_d = nc.dram_tensor("ig_wT", [2 * H, E], F32, kind="ExternalInput")
    ig_b_d = nc.dram_tensor("ig_b", [E], F32, kind="ExternalInput")
    tg_wT_d = nc.dram_tensor("tg_wT", [2 * H, E], F32, kind="ExternalInput")
    tg_b_d = nc.dram_tensor("tg_b", [E], F32, kind="ExternalInput")
    w1T_d = nc.dram_tensor("w1T", [E, H, I], F32, kind="ExternalInput")
    b1_d = nc.dram_tensor("b1", [E, I], F32, kind="ExternalInput")
    w2T_d = nc.dram_tensor("w2T", [E, I, H], F32, kind="ExternalInput")
    out_q_d = nc.dram_tensor("out_q", [SQ, H], F32, kind="ExternalOutput")
    out_img_d = nc.dram_tensor("out_img", [SI, H], F32, kind="ExternalOutput")

    def bc_ap(vec_ap, n):
        # [n] DRAM vector -> [128, n] partition-broadcast read
        return bass.AP(tensor=vec_ap.tensor, offset=vec_ap.offset,
                       ap=[[0, P], [1, n]])

    def row_ap(vec_ap, n):
        return bass.AP(tensor=vec_ap.tensor, offset=vec_ap.offset,
                       ap=[[0, 1], [1, n]])

    with tile.TileContext(nc) as tc, ExitStack() as ctx:
        consts = ctx.enter_context(tc.tile_pool(name="consts", bufs=1))
        persist = ctx.enter_context(tc.tile_pool(name="persist", bufs=1))
        psum = ctx.enter_context(tc.tile_pool(name="psum", bufs=1, space="PSUM"))

        def ps_mm(name):
            return psum.tile([P, 512], F32, tag="mm", bufs=3, name=name)

        def ps_tr(name):
            return psum.tile([P, P], F32, tag="tr", bufs=2, name=name)

        def ps_ctx(name):
            return psum.tile([P, DH + 2], F32, tag="ctx", bufs=2, name=name)

        def ps_sm(name):
            return psum.tile([P, E], F32, tag="sm", bufs=1, name=name)

        ident = consts.tile([P, P], F32)
        make_identity(nc, ident)
        ones2_f32 = consts.tile([P, 2], F32)
        nc.vector.memset(ones2_f32, 1.0)
        ones_f32_row = consts.tile([1, P], F32)
        nc.vector.memset(ones_f32_row, 1.0)
        ones_row = consts.tile([1, P], F32R)
        nc.vector.tensor_copy(out=ones_row, in_=ones_f32_row)
        eps_col = consts.tile([P, 1], F32)
        nc.vector.memset(eps_col, EPS)

        ln_cols = {}
        for nm, d in ln_d.items():
            t = consts.tile([P, HC], F32, name=nm)
            nc.sync.dma_start(out=t, in_=d[:].rearrange("(c p) -> p c", p=P))
            ln_cols[nm] = t

        bqkv_sa = consts.tile([P, 3 * HC], F32)
        nc.sync.dma_start(out=bqkv_sa, in_=bsa_qkv_d[:].rearrange("(c p) -> p c", p=P))
        bqkv_ca = consts.tile([P, 3 * HC], F32)
        nc.sync.dma_start(out=bqkv_ca, in_=bca_qkv_d[:].rearrange("(c p) -> p c", p=P))
        bv_sa_bc = consts.tile([P, H], F32)
        nc.gpsimd.dma_start(out=bv_sa_bc, in_=bc_ap(bsa_qkv_d[2 * H:3 * H], H))
        bv_ca_bc = consts.tile([P, H], F32)
        nc.gpsimd.dma_start(out=bv_ca_bc, in_=bc_ap(bca_qkv_d[2 * H:3 * H], H))
        bo_sa_bc = consts.tile([P, H], F32)
        nc.gpsimd.dma_start(out=bo_sa_bc, in_=bc_ap(bsa_out_d[:], H))
        bo_ca_bc = consts.tile([P, H], F32)
        nc.gpsimd.dma_start(out=bo_ca_bc, in_=bc_ap(bca_out_d[:], H))

        ig_wT = consts.tile([P, 2 * HC, E], F32R)
        nc.sync.dma_start(out=ig_wT, in_=r(ig_wT_d[:].rearrange("(c p) e -> p c e", p=P)))
        tg_wT = consts.tile([P, 2 * HC, E], F32R)
        nc.sync.dma_start(out=tg_wT, in_=r(tg_wT_d[:].rearrange("(c p) e -> p c e", p=P)))
        igb_row = consts.tile([1, E], F32)
        nc.sync.dma_start(out=igb_row, in_=row_ap(ig_b_d[:], E))
        tgb_row = consts.tile([1, E], F32)
        nc.sync.dma_start(out=tgb_row, in_=row_ap(tg_b_d[:], E))

        # persistent activations
        image_tm = persist.tile([P, SIC, H], F32)
        nc.sync.dma_start(out=image_tm, in_=xi_d[:].rearrange("(c p) h -> p c h", p=P))
        image_fm = persist.tile([P, HC, SI], F32R)
        qt_tm = persist.tile([P, SQC, H], F32)
        qt2_tm = persist.tile([P, SQC, H], F32)
        probs = persist.tile([P, SAC, E], F32)

        # ---------------- helpers ----------------
        def transpose_128(dst_ap, src_ap, name, engine):
            pt = ps_tr(name)
            nc.tensor.transpose(pt, src_ap, ident)
            if engine is nc.scalar:
                nc.scalar.copy(out=dst_ap, in_=pt)
            else:
                nc.vector.tensor_copy(out=dst_ap, in_=pt)

        def ln_to_fm(pool, x_tm, n_chunks, g_cols, b_cols, out_fm, tag):
            """LayerNorm over features of x_tm [P, n_chunks, H]; normalized +
            transposed + gamma/beta into out_fm [P, HC, n_chunks*P]."""
            xn_tm = pool.tile([P, n_chunks, H], F32, tag="ln_xn", bufs=1,
                              name=f"xn_{tag}")
            for sc in range(n_chunks):
                st6 = pool.tile([P, 6], F32, tag="bnst", bufs=3)
                mv = pool.tile([P, 2], F32, tag="bnmv", bufs=3)
                nc.vector.bn_stats(out=st6, in_=x_tm[:, sc, :])
                nc.vector.bn_aggr(out=mv, in_=st6)
                rstd = pool.tile([P, 1], F32, tag="rstd", bufs=3)
                nc.scalar.activation(out=rstd, in_=mv[:, 1:2], func=AF.Sqrt,
                                     bias=eps_col)
                nc.vector.reciprocal(out=rstd, in_=rstd)
                nc.vector.tensor_scalar(out=xn_tm[:, sc, :], in0=x_tm[:, sc, :],
                                        scalar1=mv[:, 0:1], scalar2=rstd,
                                        op0=ALU.subtract, op1=ALU.mult)
            for sc in range(n_chunks):
                for hc in range(HC):
                    pt = ps_tr(f"lnT_{tag}_{sc}_{hc}")
                    nc.tensor.transpose(pt, xn_tm[:, sc, hc * P:(hc + 1) * P],
                                        ident)
                    nc.vector.tensor_scalar(
                        out=out_fm[:, hc, sc * P:(sc + 1) * P], in0=pt,
                        scalar1=g_cols[:, hc:hc + 1],
                        scalar2=b_cols[:, hc:hc + 1],
                        op0=ALU.mult, op1=ALU.add)

        def attention(pool, x_norm_fm, kv_fm, wqkvT, bcols, bv_bc,
                      resid_tm, bo_bc, wout_d, out_tm, skc_n, psb_bufs, tag):
            """MHA: queries from x_norm_fm [P,HC,SQ]; K/V from kv_fm
            [P,HC,skc_n*P]; out_tm [P,SQC,H] = resid + bo + proj(ctx)."""
            SK = skc_n * P
            q_fm = pool.tile([P, HC, SQ], F32R, tag="qfm", name=f"qfm_{tag}")
            k_fm = pool.tile([P, HC, SK], F32R, tag="kfm", name=f"kfm_{tag}")
            v_aug = pool.tile([P, skc_n, NH, DH + 2], F32R, tag="vaug",
                              name=f"vaug_{tag}")
            for skc in range(skc_n):
                for h in range(NH):
                    nc.vector.tensor_copy(out=v_aug[:, skc, h, DH:DH + 2],
                                          in_=ones2_f32)
            for oc in range(HC):
                pq = ps_mm(f"q_{tag}_{oc}")
                for hc in range(HC):
                    nc.tensor.matmul(
                        pq, r(wqkvT[:, hc, oc * P:(oc + 1) * P]),
                        r(x_norm_fm[:, hc, :]),
                        start=(hc == 0), stop=(hc == HC - 1))
                nc.scalar.activation(out=q_fm[:, oc, :], in_=pq,
                                     func=AF.Identity,
                                     bias=bcols[:, oc:oc + 1])
            for oc in range(HC):
                for sh in range(SK // 512):
                    pk = ps_mm(f"k_{tag}_{oc}_{sh}")
                    for hc in range(HC):
                        nc.tensor.matmul(
                            pk, r(wqkvT[:, hc, H + oc * P: H + (oc + 1) * P]),
                            r(kv_fm[:, hc, sh * 512:(sh + 1) * 512]),
                            start=(hc == 0), stop=(hc == HC - 1))
                    nc.scalar.activation(
                        out=k_fm[:, oc, sh * 512:(sh + 1) * 512], in_=pk,
                        func=AF.Identity,
                        bias=bcols[:, HC + oc:HC + oc + 1])
            for skc in range(skc_n):
                pv = ps_mm(f"v_{tag}_{skc}")
                for hc in range(HC):
                    nc.tensor.matmul(
                        pv, r(kv_fm[:, hc, skc * P:(skc + 1) * P]),
                        r(wqkvT[:, hc, 2 * H:3 * H]),
                        start=(hc == 0), stop=(hc == HC - 1))
                for h in range(NH):
                    nc.vector.tensor_add(
                        out=v_aug[:, skc, h, 0:DH],
                        in0=pv[:, h * DH:(h + 1) * DH],
                        in1=bv_bc[:, h * DH:(h + 1) * DH])
            ctx_tm = pool.tile([P, SQC, H], F32, tag="ctxtm",
                               name=f"ctm_{tag}")
            for h in range(NH):
                oc, off = h // 2, (h % 2) * DH
                p_sb = pool.tile([P, skc_n, SQ], F32R, tag="psb",
                                 bufs=psb_bufs, name=f"psb_{tag}_{h}")
                for skc in range(skc_n):
                    pst = ps_mm(f"st_{tag}_{h}_{skc}")
                    nc.tensor.matmul(
                        pst,
                        r(k_fm[off:off + DH, oc, skc * P:(skc + 1) * P]),
                        r(q_fm[off:off + DH, oc, :]),
                        start=True, stop=True)
                    nc.scalar.activation(out=p_sb[:, skc, :], in_=pst,
                                         func=AF.Exp,
                                         scale=float(1.0 / np.sqrt(DH)))
                for sqc in range(SQC):
                    pc = ps_ctx(f"ctx_{tag}_{h}_{sqc}")
                    for skc in range(skc_n):
                        nc.tensor.matmul(
                            pc, r(p_sb[:, skc, sqc * P:(sqc + 1) * P]),
                            r(v_aug[:, skc, h, :]),
                            start=(skc == 0), stop=(skc == skc_n - 1))
                    rec = pool.tile([P, 1], F32, tag="rec", bufs=3)
                    nc.vector.reciprocal(out=rec, in_=pc[:, DH:DH + 1])
                    nc.vector.tensor_scalar_mul(
                        out=ctx_tm[:, sqc, h * DH:(h + 1) * DH],
                        in0=pc[:, 0:DH], scalar1=rec)
            ctx_fm = x_norm_fm  # alias: x_norm fully consumed by q/k/v
            for sc in range(SQC):
                for hc in range(HC):
                    transpose_128(ctx_fm[:, hc, sc * P:(sc + 1) * P],
                                  ctx_tm[:, sc, hc * P:(hc + 1) * P],
                                  f"cT_{tag}_{sc}_{hc}", nc.scalar)
            wo = pool.tile([P, HC, H], F32R, tag="wo", name=f"wo_{tag}")
            nc.sync.dma_start(out=wo,
                              in_=r(wout_d[:].rearrange("(c p) o -> p c o", p=P)))
            for sc in range(SQC):
                po = ps_mm(f"o_{tag}_{sc}")
                for hc in range(HC):
                    nc.tensor.matmul(
                        po, r(ctx_fm[:, hc, sc * P:(sc + 1) * P]),
                        r(wo[:, hc, :]),
                        start=(hc == 0), stop=(hc == HC - 1))
                nc.vector.tensor_add(out=out_tm[:, sc, :], in0=po,
                                     in1=resid_tm[:, sc, :])
                nc.vector.tensor_add(out=out_tm[:, sc, :],
                                     in0=out_tm[:, sc, :], in1=bo_bc)
        def gate_probs(pool, x_fm, n_chunks, chunk0, g_wT, const_row):
            """logits[s,e] = x_fm-contract g_wT[:512] + const_row; softmax
            into probs[:, chunk0+i, :]."""
            for sc in range(n_chunks):
                pl = ps_sm(f"gl_{chunk0}_{sc}")
                for hc in range(HC):
                    nc.tensor.matmul(
                        pl, r(x_fm[:, hc, sc * P:(sc + 1) * P]),
                        r(g_wT[:, hc, :]),
                        start=(hc == 0), stop=False)
                nc.tensor.matmul(pl, r(ones_row), r(const_row),
                                 start=False, stop=True)
                ex = pool.tile([P, E], F32, tag="gex", bufs=2)
                den = pool.tile([P, 1], F32, tag="gden", bufs=2)
                nc.scalar.activation(out=ex, in_=pl, func=AF.Exp,
                                     accum_out=den)
                rec = pool.tile([P, 1], F32, tag="grec", bufs=2)
                nc.vector.reciprocal(out=rec, in_=den)
                nc.vector.tensor_scalar_mul(out=probs[:, chunk0 + sc, :],
                                            in0=ex, scalar1=rec)

        # ================ phase 1: self-attention ================
        with tc.tile_pool(name="ph_sa", bufs=1) as pl:
            xq_tm = pl.tile([P, SQC, H], F32, tag="xq")
            nc.sync.dma_start(out=xq_tm,
                              in_=xq_d[:].rearrange("(c p) h -> p c h", p=P))
            wqkv_sa = pl.tile([P, HC, 3 * H], F32R, tag="wqkv")
            nc.sync.dma_start(
                out=wqkv_sa,
                in_=r(wsa_qkvT_d[:].rearrange("(c p) o -> p c o", p=P)))
            xn_fm = pl.tile([P, HC, SQ], F32R, tag="xnfm")
            ln_to_fm(pl, xq_tm, SQC, ln_cols["lnq_g"], ln_cols["lnq_b"],
                     xn_fm, "sa")
            attention(pl, xn_fm, xn_fm, wqkv_sa, bqkv_sa, bv_sa_bc, xq_tm,
                      bo_sa_bc, wsa_outT_d, qt_tm, SQC, 2, "sa")

        # ================ phase 2: cross-attention ================
        with tc.tile_pool(name="ph_ca", bufs=1) as pl:
            for sc in range(SIC):
                for hc in range(HC):
                    transpose_128(image_fm[:, hc, sc * P:(sc + 1) * P],
                                  image_tm[:, sc, hc * P:(hc + 1) * P],
                                  f"imT_{sc}_{hc}", nc.vector)
            wqkv_ca = pl.tile([P, HC, 3 * H], F32R, tag="wqkv")
            nc.sync.dma_start(
                out=wqkv_ca,
                in_=r(wca_qkvT_d[:].rearrange("(c p) o -> p c o", p=P)))
            cn_fm = pl.tile([P, HC, SQ], F32R, tag="xnfm")
            ln_to_fm(pl, qt_tm, SQC, ln_cols["lnc_g"], ln_cols["lnc_b"],
                     cn_fm, "ca")
            attention(pl, cn_fm, image_fm, wqkv_ca, bqkv_ca, bv_ca_bc, qt_tm,
                      bo_ca_bc, wca_outT_d, qt2_tm, SIC, 1, "ca")

        # ================ phase 3: gating ================
        with tc.tile_pool(name="ph_gate", bufs=1) as pl:
            imgctx = pl.tile([P, HC], F32R, tag="ictx")
            with nc.allow_low_precision(reason="gate ctx means, f32r rounding"):
                for hc in range(HC):
                    nc.vector.reduce_sum(out=imgctx[:, hc:hc + 1],
                                         in_=image_fm[:, hc, :].bitcast(F32),
                                         axis=AX.X)
            nc.scalar.activation(out=imgctx, in_=imgctx, func=AF.Copy,
                                 scale=1.0 / SI)
            xt_tm = pl.tile([P, STC, H], F32, tag="xt")
            nc.sync.dma_start(out=xt_tm,
                              in_=xt_d[:].rearrange("(c p) h -> p c h", p=P))
            text_fm = pl.tile([P, HC, ST], F32, tag="textfm")
            for sc in range(STC):
                for hc in range(HC):
                    transpose_128(text_fm[:, hc, sc * P:(sc + 1) * P],
                                  xt_tm[:, sc, hc * P:(hc + 1) * P],
                                  f"txT_{sc}_{hc}", nc.vector)
            txtctx = pl.tile([P, HC], F32R, tag="tctx")
            with nc.allow_low_precision(reason="gate ctx means, f32r rounding"):
                for hc in range(HC):
                    nc.vector.reduce_sum(out=txtctx[:, hc:hc + 1],
                                         in_=text_fm[:, hc, :], axis=AX.X)
            nc.scalar.activation(out=txtctx, in_=txtctx, func=AF.Copy,
                                 scale=1.0 / ST)
            pct = ps_sm("c_txt")
            for hc in range(HC):
                nc.tensor.matmul(pct[0:1, :], r(imgctx[:, hc:hc + 1]),
                                 r(tg_wT[:, HC + hc, :]),
                                 start=(hc == 0), stop=(hc == HC - 1))
            const_txt = pl.tile([1, E], F32R, tag="ctxt")
            nc.vector.tensor_add(out=const_txt, in0=pct[0:1, :], in1=tgb_row)
            pci = ps_sm("c_img")
            for hc in range(HC):
                nc.tensor.matmul(pci[0:1, :], r(txtctx[:, hc:hc + 1]),
                                 r(ig_wT[:, HC + hc, :]),
                                 start=(hc == 0), stop=(hc == HC - 1))
            const_img = pl.tile([1, E], F32R, tag="cimg")
            nc.vector.tensor_add(out=const_img, in0=pci[0:1, :], in1=igb_row)
            qt2_fm = pl.tile([P, HC, SQ], F32R, tag="qt2fm")
            for sc in range(SQC):
                for hc in range(HC):
                    transpose_128(qt2_fm[:, hc, sc * P:(sc + 1) * P],
                                  qt2_tm[:, sc, hc * P:(hc + 1) * P],
                                  f"q2T_{sc}_{hc}", nc.vector)
            gate_probs(pl, qt2_fm, SQC, 0, tg_wT, const_txt)
            gate_probs(pl, image_fm, SIC, SQC, ig_wT, const_img)

        # ================ phase 4: MoE ================
        with tc.tile_pool(name="ph_moe", bufs=1) as pl:
            yacc = pl.tile([P, SAC, H], F32, tag="yacc")
            qffn_fm = pl.tile([P, HC, SQ], F32R, tag="qffnfm")
            ln_to_fm(pl, qt2_tm, SQC, ln_cols["lnf_g"], ln_cols["lnf_b"],
                     qffn_fm, "f")
            seg_x = [qffn_fm[:, :, :], image_fm[:, :, 0:512],
                     image_fm[:, :, 512:1024]]
            for e in range(E):
                w1t = pl.tile([P, HC, I], F32R, tag="w1t", bufs=2,
                              name=f"w1_{e}")
                nc.sync.dma_start(
                    out=w1t, in_=r(w1T_d[e].rearrange("(c p) i -> p c i", p=P)))
                w2t = pl.tile([P, IC, H], F32R, tag="w2t", bufs=1,
                              name=f"w2_{e}")
                nc.sync.dma_start(
                    out=w2t, in_=r(w2T_d[e].rearrange("(c p) h -> p c h", p=P)))
                b1c = pl.tile([P, IC], F32, tag="b1c", bufs=2, name=f"b1_{e}")
                nc.sync.dma_start(
                    out=b1c, in_=b1_d[e].rearrange("(c p) -> p c", p=P))
                for seg in range(3):
                    xf = seg_x[seg]
                    h1 = pl.tile([P, IC, 512], F32R, tag="h1", bufs=1,
                                 name=f"h1_{e}_{seg}")
                    for ic in range(IC):
                        ph1 = ps_mm(f"h1_{e}_{seg}_{ic}")
                        for hc in range(HC):
                            nc.tensor.matmul(
                                ph1, r(w1t[:, hc, ic * P:(ic + 1) * P]),
                                r(xf[:, hc, :]),
                                start=(hc == 0), stop=(hc == HC - 1))
                        nc.scalar.activation(out=h1[:, ic, :], in_=ph1,
                                             func=AF.Gelu,
                                             bias=b1c[:, ic:ic + 1])
                    for sc4 in range(4):
                        scg = seg * 4 + sc4
                        py = ps_mm(f"y_{e}_{scg}")
                        for ic in range(IC):
                            nc.tensor.matmul(
                                py, r(h1[:, ic, sc4 * P:(sc4 + 1) * P]),
                                r(w2t[:, ic, :]),
                                start=(ic == 0), stop=(ic == IC - 1))
                        if e == 0:
                            resid = (qt2_tm[:, scg, :] if seg == 0
                                     else image_tm[:, scg - 4, :])
                            nc.vector.scalar_tensor_tensor(
                                out=yacc[:, scg, :], in0=py,
                                scalar=probs[:, scg, e:e + 1], in1=resid,
                                op0=ALU.mult, op1=ALU.add)
                        else:
                            nc.vector.scalar_tensor_tensor(
                                out=yacc[:, scg, :], in0=py,
                                scalar=probs[:, scg, e:e + 1],
                                in1=yacc[:, scg, :],
                                op0=ALU.mult, op1=ALU.add)

            for sc in range(SQC):
                nc.sync.dma_start(out=out_q_d[sc * P:(sc + 1) * P, :],
                                  in_=yacc[:, sc, :])
            for sc in range(SIC):
                nc.sync.dma_start(out=out_img_d[sc * P:(sc + 1) * P, :],
                                  in_=yacc[:, SQC + sc, :])
    nc.finalize()
    return nc


def _prep_in_maps(inputs):
    f = lambda x: np.ascontiguousarray(np.asarray(x), dtype=np.float32)
    sa_in_w = f(inputs["sa_in_w"])
    ca_in_w = f(inputs["ca_in_w"])
    shared = {
        "lnq_g": f(inputs["ln_q_g"]), "lnq_b": f(inputs["ln_q_b"]),
        "lnc_g": f(inputs["ln_c_g"]), "lnc_b": f(inputs["ln_c_b"]),
        "lnf_g": f(inputs["ln_f_g"]), "lnf_b": f(inputs["ln_f_b"]),
        "wsa_qkvT": f(sa_in_w.T), "bsa_qkv": f(inputs["sa_in_b"]),
        "wsa_outT": f(np.asarray(inputs["sa_out_w"]).T),
        "bsa_out": f(inputs["sa_out_b"]),
        "wca_qkvT": f(ca_in_w.T), "bca_qkv": f(inputs["ca_in_b"]),
        "wca_outT": f(np.asarray(inputs["ca_out_w"]).T),
        "bca_out": f(inputs["ca_out_b"]),
        "ig_wT": f(np.asarray(inputs["ig_w"]).T),
        "ig_b": f(inputs["ig_b"]),
        "tg_wT": f(np.asarray(inputs["tg_w"]).T),
        "tg_b": f(inputs["tg_b"]),
        "w1T": f(np.asarray(inputs["e_w1"]).transpose(0, 2, 1)),
        "b1": f(inputs["e_b1"]),
        "w2T": f(np.asarray(inputs["e_w2"]).transpose(0, 2, 1)),
    }
    xq = f(inputs["query_tokens"])
    xi = f(inputs["image_tokens"])
    xt = f(inputs["text_context"])
    return [dict(shared, xq=xq[b], xi=xi[b], xt=xt[b]) for b in range(B)]


def run(inputs, trace=False):
    in_maps = _prep_in_maps(inputs)
    nc = build_nc()
    res = run_bass_kernel_spmd(nc, in_maps, core_ids=list(range(B)),
                               trace=trace)
    qt = np.stack([res.results[b]["out_q"] for b in range(B)])
    img = np.stack([res.results[b]["out_img"] for b in range(B)])
    return (qt, img), res


def kernel(**inputs):
    out, _ = run(inputs, trace=False)
    return out


# revision 19
# speedup vs baseline: 57.2767x; 57.2767x over previous
"""Trainium2 Bass kernel for nn_CrossModalMoELayer.

Sharding: data-parallel over batch B=8 -> one batch element per NeuronCore.
Each core runs the full layer (self-attn, cross-attn, gating, dense MoE)
for its batch element; weights replicated, pre-transposed on host so the
PE gets stationary/moving operands in natural layout.

Layouts per core:
  tm = token-major  [tokens on partitions, features on free]
  fm = feature-major[features on partitions, tokens on free]
Matmuls contract over the partition dim -> feature contractions use fm
activations; per-token scalars (LN stats, softmax denoms, expert probs)
apply in tm as per-partition scalars. Attention softmax runs max-free
(logit range is ~[-2,2]); the denominator falls out of the attn@V matmul
via a ones-column appended to V. MoE expert mixture accumulates with one
fused scalar_tensor_tensor per (expert, token-chunk); the residual is
folded in at expert 0.
"""

import numpy as np
from contextlib import ExitStack

import concourse.bass as bass
import concourse.bacc as bacc
import concourse.tile as tile
from concourse import mybir
from concourse.masks import make_identity
from concourse.bass_utils import run_bass_kernel_spmd

F32 = mybir.dt.float32
F32R = mybir.dt.float32r
AF = mybir.ActivationFunctionType
ALU = mybir.AluOpType
AX = mybir.AxisListType

B, SQ, SI, ST = 8, 512, 1024, 256
H, NH, E, I = 512, 8, 8, 1024
DH = H // NH          # 64
EPS = 1e-5
P = 128
HC = H // P           # 4
SQC = SQ // P         # 4
SIC = SI // P         # 8
STC = ST // P         # 2
IC = I // P           # 8
SAC = (SQ + SI) // P  # 12


def r(ap):
    return ap.bitcast(F32R)


def build_nc(trace_sim=False):
    nc = bacc.Bacc("TRN2", target_bir_lowering=False, debug=False,
                   enable_asserts=False)

    xq_d = nc.dram_tensor("xq", [SQ, H], F32, kind="ExternalInput")
    xi_d = nc.dram_tensor("xi", [SI, H], F32, kind="ExternalInput")
    xt_d = nc.dram_tensor("xt", [ST, H], F32, kind="ExternalInput")
    ln_d = {}
    for nm in ["lnq_g", "lnq_b", "lnc_g", "lnc_b", "lnf_g", "lnf_b"]:
        ln_d[nm] = nc.dram_tensor(nm, [H], F32, kind="ExternalInput")
    wsa_qkvT_d = nc.dram_tensor("wsa_qkvT", [H, 3 * H], F32, kind="ExternalInput")
    bsa_qkv_d = nc.dram_tensor("bsa_qkv", [3 * H], F32, kind="ExternalInput")
    wsa_outT_d = nc.dram_tensor("wsa_outT", [H, H], F32, kind="ExternalInput")
    bsa_out_d = nc.dram_tensor("bsa_out", [H], F32, kind="ExternalInput")
    wca_qkvT_d = nc.dram_tensor("wca_qkvT", [H, 3 * H], F32, kind="ExternalInput")
    bca_qkv_d = nc.dram_tensor("bca_qkv", [3 * H], F32, kind="ExternalInput")
    wca_outT_d = nc.dram_tensor("wca_outT", [H, H], F32, kind="ExternalInput")
    bca_out_d = nc.dram_tensor("bca_out", [H], F32, kind="ExternalInput")
    ig_wT_d = nc.dram_tensor("ig_wT", [2 * H, E], F32, kind="ExternalInput")
    ig_b_d = nc.dram_tensor("ig_b", [E], F32, kind="ExternalInput")
    tg_wT_d = nc.dram_tensor("tg_wT", [2 * H, E], F32, kind="ExternalInput")
    tg_b_d = nc.dram_tensor("tg_b", [E], F32, kind="ExternalInput")
    w1T_d = nc.dram_tensor("w1T", [E, H, I], F32, kind="ExternalInput")
    b1_d = nc.dram_tensor("b1", [E, I], F32, kind="ExternalInput")
    w2T_d = nc.dram_tensor("w2T", [E, I, H], F32, kind="ExternalInput")
    out_q_d = nc.dram_tensor("out_q", [SQ, H], F32, kind="ExternalOutput")
    out_img_d = nc.dram_tensor("out_img", [SI, H], F32, kind="ExternalOutput")

    def bc_ap(vec_ap, n):
        # [n] DRAM vector -> [128, n] partition-broadcast read
        return bass.AP(tensor=vec_ap.tensor, offset=vec_ap.offset,
                       ap=[[0, P], [1, n]])

    def row_ap(vec_ap, n):
        return bass.AP(tensor=vec_ap.tensor, offset=vec_ap.offset,
                       ap=[[0, 1], [1, n]])

    with tile.TileContext(nc, trace_sim=trace_sim) as tc, ExitStack() as ctx:
        consts = ctx.enter_context(tc.tile_pool(name="consts", bufs=1))
        persist = ctx.enter_context(tc.tile_pool(name="persist", bufs=1))
        psum = ctx.enter_context(tc.tile_pool(name="psum", bufs=1, space="PSUM"))

        def ps_mm(name):
            return psum.tile([P, 512], F32, tag="mm", bufs=4, name=name)

        def ps_tr(name):
            return psum.tile([P, P], F32, tag="tr", bufs=2, name=name)

        def ps_ctx(name):
            return psum.tile([P, DH + 2], F32, tag="ctx", bufs=2, name=name)

        def ps_sm(name):
            return psum.tile([P, DH + 2], F32, tag="ctx", bufs=2,
                             name=name)[:, 0:E]

        ident = consts.tile([P, P], F32)
        make_identity(nc, ident)
        ones2_f32 = consts.tile([P, 2], F32)
        nc.vector.memset(ones2_f32, 1.0)
        ones_f32_row = consts.tile([1, P], F32)
        nc.vector.memset(ones_f32_row, 1.0)
        ones_row = consts.tile([1, P], F32R)
        nc.vector.tensor_copy(out=ones_row, in_=ones_f32_row)
        eps_col = consts.tile([P, 1], F32)
        nc.vector.memset(eps_col, EPS)

        ln_cols = {}
        for nm, d in ln_d.items():
            t = consts.tile([P, HC], F32, name=nm)
            nc.sync.dma_start(out=t, in_=d[:].rearrange("(c p) -> p c", p=P))
            ln_cols[nm] = t

        bqkv_sa = consts.tile([P, 3 * HC], F32)
        nc.sync.dma_start(out=bqkv_sa, in_=bsa_qkv_d[:].rearrange("(c p) -> p c", p=P))
        bqkv_ca = consts.tile([P, 3 * HC], F32)
        nc.sync.dma_start(out=bqkv_ca, in_=bca_qkv_d[:].rearrange("(c p) -> p c", p=P))
        bv_sa_bc = consts.tile([P, H], F32)
        nc.gpsimd.dma_start(out=bv_sa_bc, in_=bc_ap(bsa_qkv_d[2 * H:3 * H], H))
        bv_ca_bc = consts.tile([P, H], F32)
        nc.gpsimd.dma_start(out=bv_ca_bc, in_=bc_ap(bca_qkv_d[2 * H:3 * H], H))
        bo_sa_bc = consts.tile([P, H], F32)
        nc.gpsimd.dma_start(out=bo_sa_bc, in_=bc_ap(bsa_out_d[:], H))
        bo_ca_bc = consts.tile([P, H], F32)
        nc.gpsimd.dma_start(out=bo_ca_bc, in_=bc_ap(bca_out_d[:], H))

        ig_wT = consts.tile([P, 2 * HC, E], F32R)
        nc.sync.dma_start(out=ig_wT, in_=r(ig_wT_d[:].rearrange("(c p) e -> p c e", p=P)))
        tg_wT = consts.tile([P, 2 * HC, E], F32R)
        nc.sync.dma_start(out=tg_wT, in_=r(tg_wT_d[:].rearrange("(c p) e -> p c e", p=P)))
        igb_row = consts.tile([1, E], F32)
        nc.sync.dma_start(out=igb_row, in_=row_ap(ig_b_d[:], E))
        tgb_row = consts.tile([1, E], F32)
        nc.sync.dma_start(out=tgb_row, in_=row_ap(tg_b_d[:], E))

        # persistent activations
        image_tm = persist.tile([P, SIC, H], F32)
        nc.sync.dma_start(out=image_tm, in_=xi_d[:].rearrange("(c p) h -> p c h", p=P))
        image_fm = persist.tile([P, HC, SI], F32R)
        qt_tm = persist.tile([P, SQC, H], F32)
        qt2_tm = persist.tile([P, SQC, H], F32)
        probs = persist.tile([P, SAC, E], F32)

        # ---------------- helpers ----------------
        def transpose_128(dst_ap, src_ap, name, engine):
            pt = ps_tr(name)
            nc.tensor.transpose(pt, src_ap, ident)
            if engine is nc.scalar:
                nc.scalar.copy(out=dst_ap, in_=pt)
            else:
                nc.vector.tensor_copy(out=dst_ap, in_=pt)

        def ln_to_fm(pool, x_tm, n_chunks, g_cols, b_cols, out_fm, tag):
            """LayerNorm over features of x_tm [P, n_chunks, H]; normalized +
            transposed + gamma/beta into out_fm [P, HC, n_chunks*P]."""
            xn_tm = pool.tile([P, n_chunks, H], F32, tag="ln_xn", bufs=1,
                              name=f"xn_{tag}")
            for sc in range(n_chunks):
                st6 = pool.tile([P, 6], F32, tag="bnst", bufs=3)
                mv = pool.tile([P, 2], F32, tag="bnmv", bufs=3)
                nc.vector.bn_stats(out=st6, in_=x_tm[:, sc, :])
                nc.vector.bn_aggr(out=mv, in_=st6)
                rstd = pool.tile([P, 1], F32, tag="rstd", bufs=3)
                nc.scalar.activation(out=rstd, in_=mv[:, 1:2], func=AF.Sqrt,
                                     bias=eps_col)
                nc.vector.reciprocal(out=rstd, in_=rstd)
                nc.vector.tensor_scalar(out=xn_tm[:, sc, :], in0=x_tm[:, sc, :],
                                        scalar1=mv[:, 0:1], scalar2=rstd,
                                        op0=ALU.subtract, op1=ALU.mult)
            for sc in range(n_chunks):
                for hc in range(HC):
                    pt = ps_tr(f"lnT_{tag}_{sc}_{hc}")
                    nc.tensor.transpose(pt, xn_tm[:, sc, hc * P:(hc + 1) * P],
                                        ident)
                    nc.vector.tensor_scalar(
                        out=out_fm[:, hc, sc * P:(sc + 1) * P], in0=pt,
                        scalar1=g_cols[:, hc:hc + 1],
                        scalar2=b_cols[:, hc:hc + 1],
                        op0=ALU.mult, op1=ALU.add)

        def attention(pool, x_norm_fm, kv_fm, wqkvT, bcols, bv_bc,
                      resid_tm, bo_bc, wout_d, out_tm, skc_n, psb_bufs, tag):
            """MHA: queries from x_norm_fm [P,HC,SQ]; K/V from kv_fm
            [P,HC,skc_n*P]; out_tm [P,SQC,H] = resid + bo + proj(ctx)."""
            SK = skc_n * P
            q_fm = pool.tile([P, HC, SQ], F32R, tag="qfm", name=f"qfm_{tag}")
            k_fm = pool.tile([P, HC, SK], F32R, tag="kfm", name=f"kfm_{tag}")
            v_aug = pool.tile([P, skc_n, NH, DH + 2], F32R, tag="vaug",
                              name=f"vaug_{tag}")
            for skc in range(skc_n):
                for h in range(NH):
                    nc.vector.tensor_copy(out=v_aug[:, skc, h, DH:DH + 2],
                                          in_=ones2_f32)
            for oc in range(HC):
                pq = ps_mm(f"q_{tag}_{oc}")
                for hc in range(HC):
                    nc.tensor.matmul(
                        pq, r(wqkvT[:, hc, oc * P:(oc + 1) * P]),
                        r(x_norm_fm[:, hc, :]),
                        start=(hc == 0), stop=(hc == HC - 1))
                nc.scalar.activation(out=q_fm[:, oc, :], in_=pq,
                                     func=AF.Identity,
                                     bias=bcols[:, oc:oc + 1])
            for oc in range(HC):
                for sh in range(SK // 512):
                    pk = ps_mm(f"k_{tag}_{oc}_{sh}")
                    for hc in range(HC):
                        nc.tensor.matmul(
                            pk, r(wqkvT[:, hc, H + oc * P: H + (oc + 1) * P]),
                            r(kv_fm[:, hc, sh * 512:(sh + 1) * 512]),
                            start=(hc == 0), stop=(hc == HC - 1))
                    nc.scalar.activation(
                        out=k_fm[:, oc, sh * 512:(sh + 1) * 512], in_=pk,
                        func=AF.Identity,
                        bias=bcols[:, HC + oc:HC + oc + 1])
            for skc in range(skc_n):
                pv = ps_mm(f"v_{tag}_{skc}")
                for hc in range(HC):
                    nc.tensor.matmul(
                        pv, r(kv_fm[:, hc, skc * P:(skc + 1) * P]),
                        r(wqkvT[:, hc, 2 * H:3 * H]),
                        start=(hc == 0), stop=(hc == HC - 1))
                for h in range(NH):
                    nc.vector.tensor_add(
                        out=v_aug[:, skc, h, 0:DH],
                        in0=pv[:, h * DH:(h + 1) * DH],
                        in1=bv_bc[:, h * DH:(h + 1) * DH])
            ctx_tm = pool.tile([P, SQC, H], F32, tag="ctxtm",
                               name=f"ctm_{tag}")
            for h in range(NH):
                oc, off = h // 2, (h % 2) * DH
                p_sb = pool.tile([P, skc_n, SQ], F32R, tag="psb",
                                 bufs=psb_bufs, name=f"psb_{tag}_{h}")
                for skc in range(skc_n):
                    pst = ps_mm(f"st_{tag}_{h}_{skc}")
                    nc.tensor.matmul(
                        pst,
                        r(k_fm[off:off + DH, oc, skc * P:(skc + 1) * P]),
                        r(q_fm[off:off + DH, oc, :]),
                        start=True, stop=True)
                    nc.scalar.activation(out=p_sb[:, skc, :], in_=pst,
                                         func=AF.Exp,
                                         scale=float(1.0 / np.sqrt(DH)))
                for sqc in range(SQC):
                    pc = ps_ctx(f"ctx_{tag}_{h}_{sqc}")
                    for skc in range(skc_n):
                        nc.tensor.matmul(
                            pc, r(p_sb[:, skc, sqc * P:(sqc + 1) * P]),
                            r(v_aug[:, skc, h, :]),
                            start=(skc == 0), stop=(skc == skc_n - 1))
                    rec = pool.tile([P, 1], F32, tag="rec", bufs=3)
                    nc.vector.reciprocal(out=rec, in_=pc[:, DH:DH + 1])
                    nc.vector.tensor_scalar_mul(
                        out=ctx_tm[:, sqc, h * DH:(h + 1) * DH],
                        in0=pc[:, 0:DH], scalar1=rec)
            ctx_fm = x_norm_fm  # alias: x_norm fully consumed by q/k/v
            for sc in range(SQC):
                for hc in range(HC):
                    transpose_128(ctx_fm[:, hc, sc * P:(sc + 1) * P],
                                  ctx_tm[:, sc, hc * P:(hc + 1) * P],
                                  f"cT_{tag}_{sc}_{hc}", nc.scalar)
            wo = pool.tile([P, HC, H], F32R, tag="wo", name=f"wo_{tag}")
            nc.sync.dma_start(out=wo,
                              in_=r(wout_d[:].rearrange("(c p) o -> p c o", p=P)))
            for sc in range(SQC):
                po = ps_mm(f"o_{tag}_{sc}")
                for hc in range(HC):
                    nc.tensor.matmul(
                        po, r(ctx_fm[:, hc, sc * P:(sc + 1) * P]),
                        r(wo[:, hc, :]),
                        start=(hc == 0), stop=(hc == HC - 1))
                nc.vector.tensor_add(out=out_tm[:, sc, :], in0=po,
                                     in1=resid_tm[:, sc, :])
                nc.vector.tensor_add(out=out_tm[:, sc, :],
                                     in0=out_tm[:, sc, :], in1=bo_bc)
        def gate_probs(pool, x_fm, n_chunks, chunk0, g_wT, const_row):
            """logits[s,e] = x_fm-contract g_wT[:512] + const_row; softmax
            into probs[:, chunk0+i, :]."""
            for sc in range(n_chunks):
                pl = ps_sm(f"gl_{chunk0}_{sc}")
                for hc in range(HC):
                    nc.tensor.matmul(
                        pl, r(x_fm[:, hc, sc * P:(sc + 1) * P]),
                        r(g_wT[:, hc, :]),
                        start=(hc == 0), stop=False)
                nc.tensor.matmul(pl, r(ones_row), r(const_row),
                                 start=False, stop=True)
                ex = pool.tile([P, E], F32, tag="gex", bufs=2)
                den = pool.tile([P, 1], F32, tag="gden", bufs=2)
                nc.scalar.activation(out=ex, in_=pl, func=AF.Exp,
                                     accum_out=den)
                rec = pool.tile([P, 1], F32, tag="grec", bufs=2)
                nc.vector.reciprocal(out=rec, in_=den)
                nc.vector.tensor_scalar_mul(out=probs[:, chunk0 + sc, :],
                                            in0=ex, scalar1=rec)

        # ================ phase 1: self-attention ================
        with tc.tile_pool(name="ph_sa", bufs=1) as pl:
            xq_tm = pl.tile([P, SQC, H], F32, tag="xq")
            nc.sync.dma_start(out=xq_tm,
                              in_=xq_d[:].rearrange("(c p) h -> p c h", p=P))
            wqkv_sa = pl.tile([P, HC, 3 * H], F32R, tag="wqkv")
            nc.sync.dma_start(
                out=wqkv_sa,
                in_=r(wsa_qkvT_d[:].rearrange("(c p) o -> p c o", p=P)))
            xn_fm = pl.tile([P, HC, SQ], F32R, tag="xnfm")
            ln_to_fm(pl, xq_tm, SQC, ln_cols["lnq_g"], ln_cols["lnq_b"],
                     xn_fm, "sa")
            attention(pl, xn_fm, xn_fm, wqkv_sa, bqkv_sa, bv_sa_bc, xq_tm,
                      bo_sa_bc, wsa_outT_d, qt_tm, SQC, 2, "sa")

        # ================ phase 2: cross-attention ================
        with tc.tile_pool(name="ph_ca", bufs=1) as pl:
            for sc in range(SIC):
                for hc in range(HC):
                    transpose_128(image_fm[:, hc, sc * P:(sc + 1) * P],
                                  image_tm[:, sc, hc * P:(hc + 1) * P],
                                  f"imT_{sc}_{hc}", nc.vector)
            wqkv_ca = pl.tile([P, HC, 3 * H], F32R, tag="wqkv")
            nc.sync.dma_start(
                out=wqkv_ca,
                in_=r(wca_qkvT_d[:].rearrange("(c p) o -> p c o", p=P)))
            cn_fm = pl.tile([P, HC, SQ], F32R, tag="xnfm")
            ln_to_fm(pl, qt_tm, SQC, ln_cols["lnc_g"], ln_cols["lnc_b"],
                     cn_fm, "ca")
            attention(pl, cn_fm, image_fm, wqkv_ca, bqkv_ca, bv_ca_bc, qt_tm,
                      bo_ca_bc, wca_outT_d, qt2_tm, SIC, 1, "ca")

        # ================ phase 3: gating ================
        with tc.tile_pool(name="ph_gate", bufs=1) as pl:
            imgctx = pl.tile([P, HC], F32R, tag="ictx")
            with nc.allow_low_precision(reason="gate ctx means, f32r rounding"):
                for hc in range(HC):
                    nc.vector.reduce_sum(out=imgctx[:, hc:hc + 1],
                                         in_=image_fm[:, hc, :].bitcast(F32),
                                         axis=AX.X)
            nc.scalar.activation(out=imgctx, in_=imgctx, func=AF.Copy,
                                 scale=1.0 / SI)
            xt_tm = pl.tile([P, STC, H], F32, tag="xt")
            nc.sync.dma_start(out=xt_tm,
                              in_=xt_d[:].rearrange("(c p) h -> p c h", p=P))
            text_fm = pl.tile([P, HC, ST], F32, tag="textfm")
            for sc in range(STC):
                for hc in range(HC):
                    transpose_128(text_fm[:, hc, sc * P:(sc + 1) * P],
                                  xt_tm[:, sc, hc * P:(hc + 1) * P],
                                  f"txT_{sc}_{hc}", nc.vector)
            txtctx = pl.tile([P, HC], F32R, tag="tctx")
            with nc.allow_low_precision(reason="gate ctx means, f32r rounding"):
                for hc in range(HC):
                    nc.vector.reduce_sum(out=txtctx[:, hc:hc + 1],
                                         in_=text_fm[:, hc, :], axis=AX.X)
            nc.scalar.activation(out=txtctx, in_=txtctx, func=AF.Copy,
                                 scale=1.0 / ST)
            pct = ps_sm("c_txt")
            for hc in range(HC):
                nc.tensor.matmul(pct[0:1, :], r(imgctx[:, hc:hc + 1]),
                                 r(tg_wT[:, HC + hc, :]),
                                 start=(hc == 0), stop=(hc == HC - 1))
            const_txt = pl.tile([1, E], F32R, tag="ctxt")
            nc.vector.tensor_add(out=const_txt, in0=pct[0:1, :], in1=tgb_row)
            pci = ps_sm("c_img")
            for hc in range(HC):
                nc.tensor.matmul(pci[0:1, :], r(txtctx[:, hc:hc + 1]),
                                 r(ig_wT[:, HC + hc, :]),
                                 start=(hc == 0), stop=(hc == HC - 1))
            const_img = pl.tile([1, E], F32R, tag="cimg")
            nc.vector.tensor_add(out=const_img, in0=pci[0:1, :], in1=igb_row)
            qt2_fm = pl.tile([P, HC, SQ], F32R, tag="qt2fm")
            for sc in range(SQC):
                for hc in range(HC):
                    transpose_128(qt2_fm[:, hc, sc * P:(sc + 1) * P],
                                  qt2_tm[:, sc, hc * P:(hc + 1) * P],
                                  f"q2T_{sc}_{hc}", nc.vector)
            gate_probs(pl, qt2_fm, SQC, 0, tg_wT, const_txt)
            gate_probs(pl, image_fm, SIC, SQC, ig_wT, const_img)

        # ================ phase 4: MoE ================
        with tc.tile_pool(name="ph_moe", bufs=1) as pl:
            yacc = pl.tile([P, SAC, H], F32, tag="yacc")
            qffn_fm = pl.tile([P, HC, SQ], F32R, tag="qffnfm")
            ln_to_fm(pl, qt2_tm, SQC, ln_cols["lnf_g"], ln_cols["lnf_b"],
                     qffn_fm, "f")
            seg_x = [qffn_fm[:, :, :], image_fm[:, :, 0:512],
                     image_fm[:, :, 512:1024]]
            for e in range(E):
                w1t = pl.tile([P, HC, I], F32R, tag="w1t", bufs=2,
                              name=f"w1_{e}")
                nc.sync.dma_start(
                    out=w1t, in_=r(w1T_d[e].rearrange("(c p) i -> p c i", p=P)))
                w2t = pl.tile([P, IC, H], F32R, tag="w2t", bufs=1,
                              name=f"w2_{e}")
                nc.sync.dma_start(
                    out=w2t, in_=r(w2T_d[e].rearrange("(c p) h -> p c h", p=P)))
                b1c = pl.tile([P, IC], F32, tag="b1c", bufs=2, name=f"b1_{e}")
                nc.sync.dma_start(
                    out=b1c, in_=b1_d[e].rearrange("(c p) -> p c", p=P))
                for seg in range(3):
                    xf = seg_x[seg]
                    h1 = pl.tile([P, IC, 512], F32R, tag="h1", bufs=1,
                                 name=f"h1_{e}_{seg}")
                    for ic in range(IC):
                        ph1 = ps_mm(f"h1_{e}_{seg}_{ic}")
                        for hc in range(HC):
                            nc.tensor.matmul(
                                ph1, r(w1t[:, hc, ic * P:(ic + 1) * P]),
                                r(xf[:, hc, :]),
                                start=(hc == 0), stop=(hc == HC - 1))
                        nc.scalar.activation(out=h1[:, ic, :], in_=ph1,
                                             func=AF.Gelu,
                                             bias=b1c[:, ic:ic + 1])
                    for sc4 in range(4):
                        scg = seg * 4 + sc4
                        py = ps_mm(f"y_{e}_{scg}")
                        for ic in range(IC):
                            nc.tensor.matmul(
                                py, r(h1[:, ic, sc4 * P:(sc4 + 1) * P]),
                                r(w2t[:, ic, :]),
                                start=(ic == 0), stop=(ic == IC - 1))
                        if e == 0:
                            resid = (qt2_tm[:, scg, :] if seg == 0
                                     else image_tm[:, scg - 4, :])
                            nc.vector.scalar_tensor_tensor(
                                out=yacc[:, scg, :], in0=py,
                                scalar=probs[:, scg, e:e + 1], in1=resid,
                                op0=ALU.mult, op1=ALU.add)
                        else:
                            nc.vector.scalar_tensor_tensor(
                                out=yacc[:, scg, :], in0=py,
                                scalar=probs[:, scg, e:e + 1],
                                in1=yacc[:, scg, :],
                                op0=ALU.mult, op1=ALU.add)

            for sc in range(SQC):
                nc.sync.dma_start(out=out_q_d[sc * P:(sc + 1) * P, :],
                                  in_=yacc[:, sc, :])
            for sc in range(SIC):
                nc.sync.dma_start(out=out_img_d[sc * P:(sc + 1) * P, :],
                                  in_=yacc[:, SQC + sc, :])
    nc.finalize()
    return nc


def _prep_in_maps(inputs):
    f = lambda x: np.ascontiguousarray(np.asarray(x), dtype=np.float32)
    sa_in_w = f(inputs["sa_in_w"])
    ca_in_w = f(inputs["ca_in_w"])
    shared = {
        "lnq_g": f(inputs["ln_q_g"]), "lnq_b": f(inputs["ln_q_b"]),
        "lnc_g": f(inputs["ln_c_g"]), "lnc_b": f(inputs["ln_c_b"]),
        "lnf_g": f(inputs["ln_f_g"]), "lnf_b": f(inputs["ln_f_b"]),
        "wsa_qkvT": f(sa_in_w.T), "bsa_qkv": f(inputs["sa_in_b"]),
        "wsa_outT": f(np.asarray(inputs["sa_out_w"]).T),
        "bsa_out": f(inputs["sa_out_b"]),
        "wca_qkvT": f(ca_in_w.T), "bca_qkv": f(inputs["ca_in_b"]),
        "wca_outT": f(np.asarray(inputs["ca_out_w"]).T),
        "bca_out": f(inputs["ca_out_b"]),
        "ig_wT": f(np.asarray(inputs["ig_w"]).T),
        "ig_b": f(inputs["ig_b"]),
        "tg_wT": f(np.asarray(inputs["tg_w"]).T),
        "tg_b": f(inputs["tg_b"]),
        "w1T": f(np.asarray(inputs["e_w1"]).transpose(0, 2, 1)),
        "b1": f(inputs["e_b1"]),
        "w2T": f(np.asarray(inputs["e_w2"]).transpose(0, 2, 1)),
    }
    xq = f(inputs["query_tokens"])
    xi = f(inputs["image_tokens"])
    xt = f(inputs["text_context"])
    return [dict(shared, xq=xq[b], xi=xi[b], xt=xt[b]) for b in range(B)]


def run(inputs, trace=False):
    in_maps = _prep_in_maps(inputs)
    nc = build_nc()
    res = run_bass_kernel_spmd(nc, in_maps, core_ids=list(range(B)),
                               trace=trace)
    qt = np.stack([res.results[b]["out_q"] for b in range(B)])
    img = np.stack([res.results[b]["out_img"] for b in range(B)])
    return (qt, img), res


def kernel(**inputs):
    out, _ = run(inputs, trace=False)
    return out
